# revision 28
# baseline (speedup 1.0000x reference)
"""Trainium2 Bass kernel for the SNN (LIF) network:

    cur1 = x.reshape(B,-1) @ W1.T + b1          (big fp32 matmul, once)
    200 sequential LIF steps on [B,1000] (layer 1), tiny matmul into 5
    outputs per step (layer 2), second LIF on [B,5].

Distribution over 8 cores (v3):
  Phase A: contraction(K)-sharded exact-fp32 matmul (fp16 hi/lo, 3
           passes), looped hidden-chunk (128 cols) OUTER / K-tile inner
           with x resident in SBUF. Each chunk's partial [256, 128]
           goes through its own ReduceScatter(add) immediately, so all
           8 collectives overlap the remaining chunks' matmuls; each is
           followed by a pipelined PE-transpose + b1 bias into the scan
           layout. Each core ends with curb [128h, (chunk, 32batch)].
  Phase B: per-core LIF layer-1 scan, hidden on partitions. One custom
           DVE instruction per step into a per-group (G=4) buffer;
           ONE batched ACT Sign per group converts the whole group's
           mems to g=sign(m-1) in fp16.
  Phase C: per group, W2 chunks (bf16 hi+lo) are the PE *stationary*
           operands (LDW is 5 cols = ~4ns) and the spikes stream as the
           moving operand; 16 accumulating matmuls -> PSUM [5, 4, 32].
           ACT drains each group into c2stage [5, T, 32] (no bias).
  Phase D: layer-2 LIF runs 2 groups behind as custom DVE ops directly
           in [5, 32] layout; the op folds the effective bias b2eff in
           via the per-partition C1 scalar. mem2rec [5, T, 32] is both
           the recurrent state chain and the recorded output. GPSIMD
           extracts spk2 in 8-group batches; outputs DMA out in blocks.
"""
import os
import sys

if "/opt/trn_rl_repo" not in sys.path:
    sys.path.insert(0, "/opt/trn_rl_repo")

# Profile every core when NTFF tracing is on: exec time = max per-core span
# with aligned starts, instead of core 0's span inflated by the runtime's
# per-device dispatch stagger while it waits at the collective.
os.environ.setdefault("BASS_PERFETTO_PROFILE_ALL_CORES", "1")

import numpy as np
import ml_dtypes

# ---------------------------------------------------------------- constants
BETA = 0.95
T = 200
B = 256
NIN = 32000
NH = 1000
NO = 5

N_CORES = 8
KPAD = 32768           # NIN padded to 256*128
KC = KPAD // N_CORES   # 4096 contraction per core
KTILES = KC // 128     # 32
HPAD = 1024            # hidden padded
BLOC = B // N_CORES    # 32 batch rows per core after ReduceScatter
NCHUNK = HPAD // 128   # 8 hidden chunks of 128
G = 4                  # group size (steps per PE batch)
NGROUP = T // G        # 50
DLAG = 2               # layer-2 group lag behind layer-1
OBLK = 8               # groups per spk2/output batch
W1SCALE = 256.0        # W1 pre-scale so the fp16 lo-half stays normal

# ---------------------------------------------------------------- custom ops
_LIF_NAME = "LIF_STEP_ANT"
_LIF2_NAME = "LIF2B_STEP_ANT"


def _register_lif_ops():
    from concourse.dve_ops import (
        DveOp, OPS, CUSTOM_DVE_SPECS, _SUB_OPCODE_FOR_NAME, _CUSTOM_DVE_ROW_BASE,
    )
    from concourse.dve_spec import Spec, Src0, Src1, C0, C1, One, lower as dve_lower, _has_src1
    from concourse.dve_uop import DveOpSpec

    def _mk(name, spec):
        for op in OPS:
            if op.name == name:
                return op
        if name not in _SUB_OPCODE_FOR_NAME:
            _SUB_OPCODE_FOR_NAME[name] = _CUSTOM_DVE_ROW_BASE + len(OPS)
        shas = {}
        for ver in ("v3", "v4"):
            s = DveOpSpec(
                name=name,
                opcode=_SUB_OPCODE_FOR_NAME[name],
                uops=dve_lower(spec, ver=ver),
                rd1_en=_has_src1(spec),
            )
            shas[ver] = s.sha(ver)
        op = DveOp(name, spec, subdim=False, uops_sha=shas)
        OPS.append(op)
        CUSTOM_DVE_SPECS[name] = op.spec
        return op

    lif = _mk(_LIF_NAME, Spec(
        body=Src0 * C0 + Src1 - (Src0 > One),
        reference=lambda in0, in1, s0: in0 * s0 + in1 - (in0 > 1.0).astype(np.float32),
    ))
    lif2 = _mk(_LIF2_NAME, Spec(
        body=Src0 * C0 + Src1 + C1 - (Src0 > One),
        reference=lambda in0, in1, s0, s1:
            in0 * s0 + in1 + s1 - (in0 > 1.0).astype(np.float32),
    ))
    return lif, lif2


# ---------------------------------------------------------------- program
_PROGRAMS = {}


def _build_program(sim=False, dbg=False):
    key = (sim, dbg)
    if key in _PROGRAMS:
        return _PROGRAMS[key]

    import concourse.bass as bass
    import concourse.tile as tile
    from concourse import bacc, mybir
    from concourse.masks import make_identity

    LIF, LIF2 = _register_lif_ops()
    f32 = mybir.dt.float32
    bf16 = mybir.dt.bfloat16
    f16 = mybir.dt.float16

    nc = bacc.Bacc("TRN2", target_bir_lowering=False, debug=False,
                   num_devices=1 if sim else N_CORES)

    # inputs (per-core)
    xth_d = nc.dram_tensor("xth", [128, KTILES, B], f16, kind="ExternalInput").ap()
    xtl_d = nc.dram_tensor("xtl", [128, KTILES, B], f16, kind="ExternalInput").ap()
    # quarter-major W1: [quarter, K-within-tile partition, KTILES, 256 hidden cols]
    w1h_d = nc.dram_tensor("w1h", [4, 128, KTILES, 256], f16, kind="ExternalInput").ap()
    w1l_d = nc.dram_tensor("w1l", [4, 128, KTILES, 256], f16, kind="ExternalInput").ap()
    b1c_d = nc.dram_tensor("b1c", [128, NCHUNK], f32, kind="ExternalInput").ap()
    w2hi_d = nc.dram_tensor("w2hi", [128, NCHUNK, NO], bf16, kind="ExternalInput").ap()
    w2lo_d = nc.dram_tensor("w2lo", [128, NCHUNK, NO], bf16, kind="ExternalInput").ap()
    b2c_d = nc.dram_tensor("b2c", [NO, 1], f32, kind="ExternalInput").ap()
    # outputs (per-core batch slice), layout (o, t, b)
    mem2_d = nc.dram_tensor("mem2rec", [NO, T, BLOC], f32, kind="ExternalOutput").ap()
    spk2_d = nc.dram_tensor("spk2rec", [NO, T, BLOC], f32, kind="ExternalOutput").ap()
    if dbg:
        curdbg_d = nc.dram_tensor("curdbg", [128, NCHUNK * BLOC], f32,
                                  kind="ExternalOutput").ap()
        c2dbg_d = nc.dram_tensor("c2dbg", [NO, T, BLOC], f32,
                                 kind="ExternalOutput").ap()

    with tile.TileContext(nc) as tc:
        with (
            tc.tile_pool(name="xres", bufs=1) as xres,
            tc.tile_pool(name="win", bufs=3) as wpool,
            tc.tile_pool(name="psA", bufs=2, space="PSUM") as psA,
            tc.tile_pool(name="cs", bufs=2) as cspool,
            tc.tile_pool(name="rsb", bufs=2) as rsbpool,
            tc.tile_pool(name="psT", bufs=2, space="PSUM") as psT,
            tc.tile_pool(name="stage", bufs=1) as stage,
            tc.tile_pool(name="dram", bufs=1, space="DRAM") as dram,
            tc.tile_pool(name="mg", bufs=3) as mpool,
            tc.tile_pool(name="gt", bufs=3) as gpool,
            tc.tile_pool(name="psC", bufs=2, space="PSUM") as psC,
            tc.tile_pool(name="sblk", bufs=2) as spool,
            tc.tile_pool(name="c2g", bufs=6) as cpool,
        ):
            # ---------------- phase A: chunk-outer K-sharded matmul
            xall_h = xres.tile([128, KTILES, B], f16, tag="xah")
            xall_l = xres.tile([128, KTILES, B], f16, tag="xal")

            # persistent small constants, loaded early
            b1t = stage.tile([128, NCHUNK], f32, tag="b1t")
            nc.sync.dma_start(b1t[:], b1c_d[:])
            ident = stage.tile([BLOC, BLOC], f32, tag="ident")
            make_identity(nc, ident[:])

            curb = stage.tile([128, NCHUNK * BLOC], f32, tag="curb")

            # hidden quarters (2 chunks each): shared x stationaries across
            # the quarter's 2 chunks keep LDWEIGHTS (2 per 6 matmuls) hidden,
            # while each quarter's ReduceScatter overlaps the next quarter's
            # matmuls.
            rs_qs = []
            for qd in range(4):
                wh = wpool.tile([128, KTILES, 256], f16, tag="wh")
                wl = wpool.tile([128, KTILES, 256], f16, tag="wl")
                if qd == 0:
                    # stage the first x tiles + first weight chunk first so
                    # the PE can start early; then the rest of x.
                    nc.sync.dma_start(xall_h[:, 0:8, :], xth_d[:, 0:8, :])
                    nc.sync.dma_start(xall_l[:, 0:8, :], xtl_d[:, 0:8, :])
                # chunked weight DMAs: matmuls for kt-range r can start as
                # soon as chunk r lands (region deps), and the WAR release on
                # the rotated buffer is finer-grained.
                for r in range(4):
                    ktr = slice(r * 8, (r + 1) * 8)
                    nc.sync.dma_start(wh[:, ktr, :], w1h_d[qd][:, ktr, :])
                    nc.sync.dma_start(wl[:, ktr, :], w1l_d[qd][:, ktr, :])
                    if qd == 0 and r < 3:
                        q = r + 1
                        nc.sync.dma_start(xall_h[:, q * 8:(q + 1) * 8, :],
                                          xth_d[:, q * 8:(q + 1) * 8, :])
                        nc.sync.dma_start(xall_l[:, q * 8:(q + 1) * 8, :],
                                          xtl_d[:, q * 8:(q + 1) * 8, :])

                # both mb accumulators packed into one PSUM bank-tile
                psq = psA.tile([128, 2, 256], f32, tag="psq")
                ps = [psq[:, mb, :] for mb in range(2)]
                for kt in range(KTILES):
                    for mb in range(2):
                        xh_s = xall_h[:, kt, mb * 128:(mb + 1) * 128]
                        xl_s = xall_l[:, kt, mb * 128:(mb + 1) * 128]
                        # start=True clears has_written for the WHOLE bank, so
                        # only the very first matmul into the packed bank-tile
                        # may carry it; the other region's first write lands on
                        # cleared bits and overwrites per the per-element rule.
                        nc.tensor.matmul(ps[mb], xh_s, wl[:, kt, :],
                                         start=(kt == 0 and mb == 0),
                                         stop=False)
                        nc.tensor.matmul(ps[mb], xh_s, wh[:, kt, :],
                                         start=False, stop=False)
                        nc.tensor.matmul(ps[mb], xl_s, wh[:, kt, :],
                                         start=False, stop=(kt == KTILES - 1))

                # drain (scale back) + store partial to DRAM, batch-major
                partial = dram.tile([B, 256], f32, tag=f"pa{qd}", name=f"partial{qd}")
                for mb in range(2):
                    csb = cspool.tile([128, 256], f32, tag=f"cs{mb}",
                                      name=f"cs{mb}_{qd}")
                    nc.scalar.activation(csb[:], ps[mb],
                                         mybir.ActivationFunctionType.Copy,
                                         scale=1.0 / W1SCALE)
                    # scalar queue: keeps the sync queue a pure input-prefetch
                    # stream (in-order queues head-of-line block otherwise)
                    nc.scalar.dma_start(partial[mb * 128:(mb + 1) * 128, :], csb[:])

                rs_q = dram.tile([BLOC, 256], f32, tag=f"rs{qd}", name=f"rs{qd}")
                if sim:
                    nc.sync.dma_start(rs_q[:], partial[0:BLOC, :])
                else:
                    nc.gpsimd.collective_compute(
                        "ReduceScatter",
                        mybir.AluOpType.add,
                        replica_groups=[list(range(N_CORES))],
                        ins=[partial.opt()],
                        outs=[rs_q.opt()],
                    )
                rs_qs.append(rs_q)

            # RS-dependent work LAST, so nothing that waits on a collective
            # sits ahead of later quarters' matmuls in any in-order queue.
            # rsb loads ride the scalar queue after all partial stores;
            # b1 folds in via a DVE tensor_scalar with a [P,1] scalar AP.
            for qd in range(4):
                rsb = rsbpool.tile([BLOC, 256], f32, tag="rsb",
                                   name=f"rsb{qd}")
                nc.scalar.dma_start(rsb[:], rs_qs[qd][:])
                for cl in range(2):
                    c = 2 * qd + cl
                    pt = psT.tile([128, BLOC], f32, tag="pst",
                                  name=f"pst{qd}{cl}")
                    nc.tensor.matmul(pt[:], rsb[:, cl * 128:(cl + 1) * 128],
                                     ident[:], start=True, stop=True,
                                     is_transpose=True)
                    nc.vector.tensor_scalar(
                        curb[:, c * BLOC:(c + 1) * BLOC], pt[:],
                        b1t[:, c:c + 1], None, mybir.AluOpType.add)

            if dbg:
                nc.sync.dma_start(curdbg_d[:], curb[:])

            # ---------------- scan constants
            w2hi_t = stage.tile([128, NCHUNK, NO], bf16, tag="w2hi")
            nc.sync.dma_start(w2hi_t[:], w2hi_d[:])
            w2lo_t = stage.tile([128, NCHUNK, NO], bf16, tag="w2lo")
            nc.sync.dma_start(w2lo_t[:], w2lo_d[:])
            b2c_t = stage.tile([NO, 1], f32, tag="b2c")
            nc.sync.dma_start(b2c_t[:], b2c_d[:])
            biasm1 = stage.tile([128, 1], f32, tag="bm1")
            nc.vector.memset(biasm1[:], -1.0)
            zeros_t = stage.tile([128, NCHUNK * BLOC], f32, tag="zeros")
            nc.vector.memset(zeros_t[:], 0.0)
            z5 = stage.tile([NO, BLOC], f32, tag="z5")
            nc.vector.memset(z5[:], 0.0)
            mem2rec = stage.tile([NO, T, BLOC], f32, tag="m2r")
            c2tiles = [None] * NGROUP  # rotating per-group cur2 tiles

            def d_step(dt):
                """Layer-2 LIF step dt (0-based) on DVE, [5, 32] layout."""
                in0 = z5[:] if dt == 0 else mem2rec[:, dt - 1, :]
                nc.vector._custom_dve(
                    LIF2,
                    out=mem2rec[:, dt, :],
                    in0=in0,
                    in1=c2tiles[dt // G][:, dt % G, :],
                    s0=BETA,
                    s1=b2c_t[:],
                )

            def out_block(t0, t1):
                # DVE, not GPSIMD: a [5, N] gpsimd tensor_scalar measures
                # ~13.7us and stalls concurrent DVE work via the shared SBUF
                # port; the DVE op is ~0.7us.
                sblk = spool.tile([NO, (OBLK + DLAG) * G, BLOC], f32, tag="sblk")
                nc.vector.tensor_scalar(
                    sblk[:, 0:t1 - t0, :], mem2rec[:, t0:t1, :], 1.0, None,
                    mybir.AluOpType.is_gt)
                nc.sync.dma_start(mem2_d[:, t0:t1, :], mem2rec[:, t0:t1, :])
                nc.sync.dma_start(spk2_d[:, t0:t1, :], sblk[:, 0:t1 - t0, :])

            # ---------------- phase B/C/D: fused scan
            mem_prev = zeros_t[:]
            mg = None
            out_done = 0
            for t in range(1, T + 1):
                gi, sl = (t - 1) // G, (t - 1) % G
                if sl == 0:
                    mg = mpool.tile([128, G, NCHUNK * BLOC], f32, tag="mg")
                out_v = mg[:, sl, :]
                nc.vector._custom_dve(LIF, out=out_v, in0=mem_prev,
                                      in1=curb[:], s0=BETA)
                mem_prev = out_v
                if sl == G - 1:
                    # batched sign for the whole group -> g in {-1,+1} bf16
                    gt = gpool.tile([128, G, NCHUNK, BLOC], bf16, tag="gt")
                    nc.scalar.activation(
                        gt[:].rearrange("p g c b -> p (g c b)"),
                        mg[:].rearrange("p g f -> p (g f)"),
                        mybir.ActivationFunctionType.Sign,
                        bias=biasm1[:], scale=1.0,
                    )
                    # layer-2 contraction: W2 chunks stationary, spikes moving
                    pc = psC.tile([NO, G, BLOC], f32, tag="pc")
                    for c2 in range(NCHUNK):
                        rhs = gt[:, :, c2, :]
                        nc.tensor.matmul(pc[:], w2hi_t[:, c2, :], rhs,
                                         start=(c2 == 0), stop=False)
                        nc.tensor.matmul(pc[:], w2lo_t[:, c2, :], rhs,
                                         start=False, stop=(c2 == NCHUNK - 1))
                    # drain current group's cur2 (sans bias) into its tile
                    c2g = cpool.tile([NO, G, BLOC], f32, tag="c2g")
                    c2tiles[gi] = c2g
                    nc.scalar.activation(
                        c2g[:], pc[:], mybir.ActivationFunctionType.Copy)
                    if dbg:
                        nc.sync.dma_start(c2dbg_d[:, gi * G:(gi + 1) * G, :],
                                          c2g[:])
                    # layer-2 LIF, lagged DLAG groups
                    if gi >= DLAG:
                        dg = gi - DLAG
                        for dt in range(dg * G, (dg + 1) * G):
                            d_step(dt)
                    # spk2 + output DMA in OBLK-group blocks, lagged
                    if gi >= DLAG + OBLK and (gi - DLAG - OBLK) % OBLK == 0:
                        blk = (gi - DLAG - OBLK) // OBLK
                        out_block(blk * OBLK * G, (blk + 1) * OBLK * G)
                        out_done = (blk + 1) * OBLK * G
                    if gi == NGROUP - 1:
                        # flush everything already d-stepped (t < (gi-1)*G)
                        out_block(out_done, (gi - DLAG + 1) * G)
                        out_done = (gi - DLAG + 1) * G

            # ---------------- tail
            for dt in range((NGROUP - DLAG) * G, T):
                d_step(dt)
            out_block(out_done, T)

    nc.compile()
    _PROGRAMS[key] = (nc, LIF, LIF2)
    return _PROGRAMS[key]


# ---------------------------------------------------------------- host prep
def _prep_inputs(x, W1, b1, W2, b2):
    f32 = np.float32
    f16 = np.float16
    x_flat = np.ascontiguousarray(x.reshape(B, -1).astype(f32, copy=False))
    xT = np.zeros((KPAD, B), f32)
    xT[:NIN] = x_flat.T
    xTh = xT.astype(f16)
    xTl = (xT - xTh.astype(f32)).astype(f16)
    w1T = np.zeros((KPAD, HPAD), f32)
    w1T[:NIN, :NH] = W1.astype(f32, copy=False).T * W1SCALE
    w1Th = w1T.astype(f16)
    w1Tl = (w1T - w1Th.astype(f32)).astype(f16)
    b1p = np.full(HPAD, -10.0, f32)
    b1p[:NH] = b1
    b1c = np.ascontiguousarray(b1p.reshape(NCHUNK, 128).T)          # [128, 8]
    W2e = np.zeros((HPAD, NO), f32)
    W2e[:NH] = 0.5 * W2.astype(f32, copy=False).T
    w2stack = np.ascontiguousarray(W2e.reshape(NCHUNK, 128, NO).transpose(1, 0, 2))
    w2hi = w2stack.astype(ml_dtypes.bfloat16)
    w2lo = (w2stack - w2hi.astype(f32)).astype(ml_dtypes.bfloat16)
    b2eff = (b2.astype(f32) + 0.5 * W2.astype(f32).sum(axis=1)).reshape(NO, 1)
    b2eff = np.ascontiguousarray(b2eff.astype(f32))

    in_maps = []
    for cidx in range(N_CORES):
        ksl = slice(cidx * KC, (cidx + 1) * KC)
        xh = xTh[ksl].reshape(KTILES, 128, B).transpose(1, 0, 2)
        xl = xTl[ksl].reshape(KTILES, 128, B).transpose(1, 0, 2)
        # [KC, HPAD] -> [KTILES, 128, 4, 256] -> [4, 128, KTILES, 256]
        wh = w1Th[ksl].reshape(KTILES, 128, 4, 256).transpose(2, 1, 0, 3)
        wl = w1Tl[ksl].reshape(KTILES, 128, 4, 256).transpose(2, 1, 0, 3)
        in_maps.append({
            "xth": np.ascontiguousarray(xh),
            "xtl": np.ascontiguousarray(xl),
            "w1h": np.ascontiguousarray(wh),
            "w1l": np.ascontiguousarray(wl),
            "b1c": b1c,
            "w2hi": w2hi,
            "w2lo": w2lo,
            "b2c": b2eff,
        })
    return in_maps


def _gather(results):
    spk_parts, mem_parts = [], []
    for r in results:
        mem_parts.append(r["mem2rec"].transpose(1, 2, 0))  # [T, BLOC, NO]
        spk_parts.append(r["spk2rec"].transpose(1, 2, 0))
    mem2 = np.concatenate(mem_parts, axis=1).astype(np.float32)  # [200, 256, 5]
    spk2 = np.concatenate(spk_parts, axis=1).astype(np.float32)
    return spk2, mem2


def run_raw(inputs, dbg=False, **kwargs):
    """Build+run; returns BassKernelResults (for profiling from test.py)."""
    from concourse.bass_utils import run_bass_kernel_spmd

    nc, _, _ = _build_program(dbg=dbg)
    in_maps = _prep_inputs(**inputs)
    return run_bass_kernel_spmd(nc, in_maps, core_ids=list(range(N_CORES)), **kwargs)


def kernel(x, W1, b1, W2, b2):
    res = run_raw(dict(x=x, W1=W1, b1=b1, W2=W2, b2=b2))
    return _gather(res.results)


if __name__ == "__main__":
    rng = np.random.default_rng(0)
    ins = {
        "x": rng.standard_normal((B, 2, 80, 200)).astype(np.float32),
        "W1": rng.uniform(-1, 1, (NH, NIN)).astype(np.float32) / np.sqrt(NIN),
        "b1": rng.uniform(-1, 1, NH).astype(np.float32) / np.sqrt(NIN),
        "W2": rng.uniform(-1, 1, (NO, NH)).astype(np.float32) / np.sqrt(NH),
        "b2": rng.uniform(-1, 1, NO).astype(np.float32) / np.sqrt(NH),
    }
    spk2, mem2 = kernel(**ins)
    print("shapes:", spk2.shape, mem2.shape, spk2.dtype, mem2.dtype)
    print("spk2 mean:", spk2.mean(), "mem2 std:", mem2.std())


# revision 30
# speedup vs baseline: 1.2177x; 1.2177x over previous
"""Trainium2 Bass kernel for the SNN (LIF) network:

    cur1 = x.reshape(B,-1) @ W1.T + b1          (big fp32 matmul, once)
    200 sequential LIF steps on [B,1000] (layer 1), tiny matmul into 5
    outputs per step (layer 2), second LIF on [B,5].

Distribution over 8 cores (v3):
  Phase A: contraction(K)-sharded exact-fp32 matmul (fp16 hi/lo, 3
           passes), looped hidden-chunk (128 cols) OUTER / K-tile inner
           with x resident in SBUF. Each chunk's partial [256, 128]
           goes through its own ReduceScatter(add) immediately, so all
           8 collectives overlap the remaining chunks' matmuls; each is
           followed by a pipelined PE-transpose + b1 bias into the scan
           layout. Each core ends with curb [128h, (chunk, 32batch)].
  Phase B: per-core LIF layer-1 scan, hidden on partitions. One custom
           DVE instruction per step into a per-group (G=4) buffer;
           ONE batched ACT Sign per group converts the whole group's
           mems to g=sign(m-1) in fp16.
  Phase C: per group, W2 chunks (bf16 hi+lo) are the PE *stationary*
           operands (LDW is 5 cols = ~4ns) and the spikes stream as the
           moving operand; 16 accumulating matmuls -> PSUM [5, 4, 32].
           ACT drains each group into c2stage [5, T, 32] (no bias).
  Phase D: layer-2 LIF runs 2 groups behind as custom DVE ops directly
           in [5, 32] layout; the op folds the effective bias b2eff in
           via the per-partition C1 scalar. mem2rec [5, T, 32] is both
           the recurrent state chain and the recorded output. GPSIMD
           extracts spk2 in 8-group batches; outputs DMA out in blocks.
"""
import os
import sys

if "/opt/trn_rl_repo" not in sys.path:
    sys.path.insert(0, "/opt/trn_rl_repo")

# Profile every core when NTFF tracing is on: exec time = max per-core span
# with aligned starts, instead of core 0's span inflated by the runtime's
# per-device dispatch stagger while it waits at the collective.
os.environ.setdefault("BASS_PERFETTO_PROFILE_ALL_CORES", "1")

import numpy as np
import ml_dtypes

# ---------------------------------------------------------------- constants
BETA = 0.95
T = 200
B = 256
NIN = 32000
NH = 1000
NO = 5

N_CORES = 8
KPAD = 32768           # NIN padded to 256*128
KC = KPAD // N_CORES   # 4096 contraction per core
KTILES = KC // 128     # 32
HPAD = 1024            # hidden padded
BLOC = B // N_CORES    # 32 batch rows per core after ReduceScatter
NCHUNK = HPAD // 128   # 8 hidden chunks of 128
G = 4                  # group size (steps per PE batch)
NGROUP = T // G        # 50
DLAG = 2               # layer-2 group lag behind layer-1
OBLK = 8               # groups per spk2/output batch
W1SCALE = 256.0        # W1 pre-scale so the fp16 lo-half stays normal

# ---------------------------------------------------------------- custom ops
_LIF_NAME = "LIF_STEP_ANT"
_LIF2_NAME = "LIF2B_STEP_ANT"


def _register_lif_ops():
    from concourse.dve_ops import (
        DveOp, OPS, CUSTOM_DVE_SPECS, _SUB_OPCODE_FOR_NAME, _CUSTOM_DVE_ROW_BASE,
    )
    from concourse.dve_spec import Spec, Src0, Src1, C0, C1, One, lower as dve_lower, _has_src1
    from concourse.dve_uop import DveOpSpec

    def _mk(name, spec):
        for op in OPS:
            if op.name == name:
                return op
        if name not in _SUB_OPCODE_FOR_NAME:
            _SUB_OPCODE_FOR_NAME[name] = _CUSTOM_DVE_ROW_BASE + len(OPS)
        shas = {}
        for ver in ("v3", "v4"):
            s = DveOpSpec(
                name=name,
                opcode=_SUB_OPCODE_FOR_NAME[name],
                uops=dve_lower(spec, ver=ver),
                rd1_en=_has_src1(spec),
            )
            shas[ver] = s.sha(ver)
        op = DveOp(name, spec, subdim=False, uops_sha=shas)
        OPS.append(op)
        CUSTOM_DVE_SPECS[name] = op.spec
        return op

    lif = _mk(_LIF_NAME, Spec(
        body=Src0 * C0 + Src1 - (Src0 > One),
        reference=lambda in0, in1, s0: in0 * s0 + in1 - (in0 > 1.0).astype(np.float32),
    ))
    lif2 = _mk(_LIF2_NAME, Spec(
        body=Src0 * C0 + Src1 + C1 - (Src0 > One),
        reference=lambda in0, in1, s0, s1:
            in0 * s0 + in1 + s1 - (in0 > 1.0).astype(np.float32),
    ))
    return lif, lif2


# ---------------------------------------------------------------- program
_PROGRAMS = {}


def _build_program(sim=False, dbg=False):
    key = (sim, dbg)
    if key in _PROGRAMS:
        return _PROGRAMS[key]

    import concourse.bass as bass
    import concourse.tile as tile
    from concourse import bacc, mybir
    from concourse.masks import make_identity

    LIF, LIF2 = _register_lif_ops()
    f32 = mybir.dt.float32
    bf16 = mybir.dt.bfloat16
    f16 = mybir.dt.float16

    nc = bacc.Bacc("TRN2", target_bir_lowering=False, debug=False,
                   num_devices=1 if sim else N_CORES)

    # inputs (per-core)
    xth_d = nc.dram_tensor("xth", [128, KTILES, B], f16, kind="ExternalInput").ap()
    xtl_d = nc.dram_tensor("xtl", [128, KTILES, B], f16, kind="ExternalInput").ap()
    # quarter-major W1: [quarter, K-within-tile partition, KTILES, 256 hidden cols]
    w1h_d = nc.dram_tensor("w1h", [4, 128, KTILES, 256], f16, kind="ExternalInput").ap()
    w1l_d = nc.dram_tensor("w1l", [4, 128, KTILES, 256], f16, kind="ExternalInput").ap()
    b1c_d = nc.dram_tensor("b1c", [128, NCHUNK], f32, kind="ExternalInput").ap()
    w2hi_d = nc.dram_tensor("w2hi", [128, NCHUNK, NO], bf16, kind="ExternalInput").ap()
    w2lo_d = nc.dram_tensor("w2lo", [128, NCHUNK, NO], bf16, kind="ExternalInput").ap()
    b2c_d = nc.dram_tensor("b2c", [NO, 1], f32, kind="ExternalInput").ap()
    # outputs (per-core batch slice), layout (o, t, b)
    mem2_d = nc.dram_tensor("mem2rec", [NO, T, BLOC], f32, kind="ExternalOutput").ap()
    spk2_d = nc.dram_tensor("spk2rec", [NO, T, BLOC], f32, kind="ExternalOutput").ap()
    if dbg:
        curdbg_d = nc.dram_tensor("curdbg", [128, NCHUNK * BLOC], f32,
                                  kind="ExternalOutput").ap()
        c2dbg_d = nc.dram_tensor("c2dbg", [NO, T, BLOC], f32,
                                 kind="ExternalOutput").ap()

    with tile.TileContext(nc) as tc:
        with (
            tc.tile_pool(name="xres", bufs=1) as xres,
            tc.tile_pool(name="win", bufs=3) as wpool,
            tc.tile_pool(name="psA", bufs=2, space="PSUM") as psA,
            tc.tile_pool(name="cs", bufs=2) as cspool,
            tc.tile_pool(name="rsb", bufs=2) as rsbpool,
            tc.tile_pool(name="psT", bufs=2, space="PSUM") as psT,
            tc.tile_pool(name="stage", bufs=1) as stage,
            tc.tile_pool(name="dram", bufs=1, space="DRAM") as dram,
            tc.tile_pool(name="mg", bufs=3) as mpool,
            tc.tile_pool(name="gt", bufs=3) as gpool,
            tc.tile_pool(name="psC", bufs=2, space="PSUM") as psC,
            tc.tile_pool(name="sblk", bufs=2) as spool,
            tc.tile_pool(name="c2g", bufs=6) as cpool,
        ):
            # ---------------- phase A: chunk-outer K-sharded matmul
            xall_h = xres.tile([128, KTILES, B], f16, tag="xah")
            xall_l = xres.tile([128, KTILES, B], f16, tag="xal")

            # persistent small constants, loaded early
            b1t = stage.tile([128, NCHUNK], f32, tag="b1t")
            nc.sync.dma_start(b1t[:], b1c_d[:])
            ident = stage.tile([BLOC, BLOC], f32, tag="ident")
            make_identity(nc, ident[:])

            curb = stage.tile([128, NCHUNK * BLOC], f32, tag="curb")

            # hidden quarters (2 chunks each): shared x stationaries across
            # the quarter's 2 chunks keep LDWEIGHTS (2 per 6 matmuls) hidden,
            # while each quarter's ReduceScatter overlaps the next quarter's
            # matmuls.
            rs_qs = []
            for qd in range(4):
                wh = wpool.tile([128, KTILES, 256], f16, tag="wh")
                wl = wpool.tile([128, KTILES, 256], f16, tag="wl")
                if qd == 0:
                    # stage the first x tiles + first weight chunk first so
                    # the PE can start early; then the rest of x.
                    nc.sync.dma_start(xall_h[:, 0:8, :], xth_d[:, 0:8, :])
                    nc.sync.dma_start(xall_l[:, 0:8, :], xtl_d[:, 0:8, :])
                # chunked weight DMAs: matmuls for kt-range r can start as
                # soon as chunk r lands (region deps), and the WAR release on
                # the rotated buffer is finer-grained.
                for r in range(4):
                    ktr = slice(r * 8, (r + 1) * 8)
                    nc.sync.dma_start(wh[:, ktr, :], w1h_d[qd][:, ktr, :])
                    nc.sync.dma_start(wl[:, ktr, :], w1l_d[qd][:, ktr, :])
                    if qd == 0 and r < 3:
                        q = r + 1
                        nc.sync.dma_start(xall_h[:, q * 8:(q + 1) * 8, :],
                                          xth_d[:, q * 8:(q + 1) * 8, :])
                        nc.sync.dma_start(xall_l[:, q * 8:(q + 1) * 8, :],
                                          xtl_d[:, q * 8:(q + 1) * 8, :])

                # both mb accumulators packed into one PSUM bank-tile
                psq = psA.tile([128, 2, 256], f32, tag="psq")
                ps = [psq[:, mb, :] for mb in range(2)]
                for kt in range(KTILES):
                    for mb in range(2):
                        xh_s = xall_h[:, kt, mb * 128:(mb + 1) * 128]
                        xl_s = xall_l[:, kt, mb * 128:(mb + 1) * 128]
                        # start=True clears has_written for the WHOLE bank, so
                        # only the very first matmul into the packed bank-tile
                        # may carry it; the other region's first write lands on
                        # cleared bits and overwrites per the per-element rule.
                        nc.tensor.matmul(ps[mb], xh_s, wl[:, kt, :],
                                         start=(kt == 0 and mb == 0),
                                         stop=False)
                        nc.tensor.matmul(ps[mb], xh_s, wh[:, kt, :],
                                         start=False, stop=False)
                        nc.tensor.matmul(ps[mb], xl_s, wh[:, kt, :],
                                         start=False, stop=(kt == KTILES - 1))

                # drain (scale back) + store partial to DRAM, batch-major
                partial = dram.tile([B, 256], f32, tag=f"pa{qd}", name=f"partial{qd}")
                for mb in range(2):
                    csb = cspool.tile([128, 256], f32, tag=f"cs{mb}",
                                      name=f"cs{mb}_{qd}")
                    nc.scalar.activation(csb[:], ps[mb],
                                         mybir.ActivationFunctionType.Copy,
                                         scale=1.0 / W1SCALE)
                    # scalar queue: keeps the sync queue a pure input-prefetch
                    # stream (in-order queues head-of-line block otherwise)
                    nc.scalar.dma_start(partial[mb * 128:(mb + 1) * 128, :], csb[:])

                rs_q = dram.tile([BLOC, 256], f32, tag=f"rs{qd}", name=f"rs{qd}")
                if sim:
                    nc.sync.dma_start(rs_q[:], partial[0:BLOC, :])
                else:
                    nc.gpsimd.collective_compute(
                        "ReduceScatter",
                        mybir.AluOpType.add,
                        replica_groups=[list(range(N_CORES))],
                        ins=[partial.opt()],
                        outs=[rs_q.opt()],
                    )
                rs_qs.append(rs_q)

            # RS-dependent work LAST, so nothing that waits on a collective
            # sits ahead of later quarters' matmuls in any in-order queue.
            # rsb loads ride the scalar queue after all partial stores;
            # b1 folds in via a DVE tensor_scalar with a [P,1] scalar AP.
            for qd in range(4):
                rsb = rsbpool.tile([BLOC, 256], f32, tag="rsb",
                                   name=f"rsb{qd}")
                nc.scalar.dma_start(rsb[:], rs_qs[qd][:])
                for cl in range(2):
                    c = 2 * qd + cl
                    pt = psT.tile([128, BLOC], f32, tag="pst",
                                  name=f"pst{qd}{cl}")
                    nc.tensor.matmul(pt[:], rsb[:, cl * 128:(cl + 1) * 128],
                                     ident[:], start=True, stop=True,
                                     is_transpose=True)
                    nc.vector.tensor_scalar(
                        curb[:, c * BLOC:(c + 1) * BLOC], pt[:],
                        b1t[:, c:c + 1], None, mybir.AluOpType.add)

            if dbg:
                nc.sync.dma_start(curdbg_d[:], curb[:])

            # ---------------- scan constants
            w2hi_t = stage.tile([128, NCHUNK, NO], bf16, tag="w2hi")
            nc.sync.dma_start(w2hi_t[:], w2hi_d[:])
            w2lo_t = stage.tile([128, NCHUNK, NO], bf16, tag="w2lo")
            nc.sync.dma_start(w2lo_t[:], w2lo_d[:])
            b2c_t = stage.tile([NO, 1], f32, tag="b2c")
            nc.sync.dma_start(b2c_t[:], b2c_d[:])
            biasm1 = stage.tile([128, 1], f32, tag="bm1")
            nc.vector.memset(biasm1[:], -1.0)
            zeros_t = stage.tile([128, NCHUNK * BLOC], f32, tag="zeros")
            nc.vector.memset(zeros_t[:], 0.0)
            z5 = stage.tile([NO, BLOC], f32, tag="z5")
            nc.vector.memset(z5[:], 0.0)
            mem2rec = stage.tile([NO, T, BLOC], f32, tag="m2r")
            c2tiles = [None] * NGROUP  # rotating per-group cur2 tiles

            def d_step(dt):
                """Layer-2 LIF step dt (0-based) on DVE, [5, 32] layout.

                b2eff is pre-folded into c2 by the drain's ACT bias, so the
                plain 1-scalar LIF op suffices (the [P,1] C1 variant costs
                ~+60ns/op in AP latch setup).
                """
                in0 = z5[:] if dt == 0 else mem2rec[:, dt - 1, :]
                nc.vector._custom_dve(
                    LIF,
                    out=mem2rec[:, dt, :],
                    in0=in0,
                    in1=c2tiles[dt // G][:, dt % G, :],
                    s0=BETA,
                )

            def out_block(t0, t1):
                # DVE, not GPSIMD: a [5, N] gpsimd tensor_scalar measures
                # ~13.7us and stalls concurrent DVE work via the shared SBUF
                # port; the DVE op is ~0.7us.
                sblk = spool.tile([NO, (OBLK + DLAG) * G, BLOC], f32, tag="sblk")
                nc.vector.tensor_scalar(
                    sblk[:, 0:t1 - t0, :], mem2rec[:, t0:t1, :], 1.0, None,
                    mybir.AluOpType.is_gt)
                nc.sync.dma_start(mem2_d[:, t0:t1, :], mem2rec[:, t0:t1, :])
                nc.sync.dma_start(spk2_d[:, t0:t1, :], sblk[:, 0:t1 - t0, :])

            # ---------------- phase B/C/D: fused scan
            mem_prev = zeros_t[:]
            mg = None
            out_done = 0
            for t in range(1, T + 1):
                gi, sl = (t - 1) // G, (t - 1) % G
                if sl == 0:
                    mg = mpool.tile([128, G, NCHUNK * BLOC], f32, tag="mg")
                out_v = mg[:, sl, :]
                nc.vector._custom_dve(LIF, out=out_v, in0=mem_prev,
                                      in1=curb[:], s0=BETA)
                mem_prev = out_v
                if sl == G - 1:
                    # batched sign for the whole group -> g in {-1,+1} bf16
                    gt = gpool.tile([128, G, NCHUNK, BLOC], bf16, tag="gt")
                    nc.scalar.activation(
                        gt[:].rearrange("p g c b -> p (g c b)"),
                        mg[:].rearrange("p g f -> p (g f)"),
                        mybir.ActivationFunctionType.Sign,
                        bias=biasm1[:], scale=1.0,
                    )
                    # layer-2 contraction: W2 chunks stationary, spikes moving
                    pc = psC.tile([NO, G, BLOC], f32, tag="pc")
                    for c2 in range(NCHUNK):
                        rhs = gt[:, :, c2, :]
                        nc.tensor.matmul(pc[:], w2hi_t[:, c2, :], rhs,
                                         start=(c2 == 0), stop=False)
                        nc.tensor.matmul(pc[:], w2lo_t[:, c2, :], rhs,
                                         start=False, stop=(c2 == NCHUNK - 1))
                    # drain current group's cur2 into its tile, folding the
                    # effective bias b2eff in via the per-partition ACT bias
                    c2g = cpool.tile([NO, G, BLOC], f32, tag="c2g")
                    c2tiles[gi] = c2g
                    nc.scalar.activation(
                        c2g[:], pc[:], mybir.ActivationFunctionType.Identity,
                        bias=b2c_t[:], scale=1.0)
                    if dbg:
                        nc.sync.dma_start(c2dbg_d[:, gi * G:(gi + 1) * G, :],
                                          c2g[:])
                    # layer-2 LIF, lagged DLAG groups
                    if gi >= DLAG:
                        dg = gi - DLAG
                        for dt in range(dg * G, (dg + 1) * G):
                            d_step(dt)
                    # spk2 + output DMA in OBLK-group blocks, lagged
                    if gi >= DLAG + OBLK and (gi - DLAG - OBLK) % OBLK == 0:
                        blk = (gi - DLAG - OBLK) // OBLK
                        out_block(blk * OBLK * G, (blk + 1) * OBLK * G)
                        out_done = (blk + 1) * OBLK * G
                    if gi == NGROUP - 1:
                        # flush everything already d-stepped (t < (gi-1)*G)
                        out_block(out_done, (gi - DLAG + 1) * G)
                        out_done = (gi - DLAG + 1) * G

            # ---------------- tail
            for dt in range((NGROUP - DLAG) * G, T):
                d_step(dt)
            out_block(out_done, T)

    nc.compile()
    _PROGRAMS[key] = (nc, LIF, LIF2)
    return _PROGRAMS[key]


# ---------------------------------------------------------------- host prep
def _prep_inputs(x, W1, b1, W2, b2):
    f32 = np.float32
    f16 = np.float16
    x_flat = np.ascontiguousarray(x.reshape(B, -1).astype(f32, copy=False))
    xT = np.zeros((KPAD, B), f32)
    xT[:NIN] = x_flat.T
    xTh = xT.astype(f16)
    xTl = (xT - xTh.astype(f32)).astype(f16)
    w1T = np.zeros((KPAD, HPAD), f32)
    w1T[:NIN, :NH] = W1.astype(f32, copy=False).T * W1SCALE
    w1Th = w1T.astype(f16)
    w1Tl = (w1T - w1Th.astype(f32)).astype(f16)
    b1p = np.full(HPAD, -10.0, f32)
    b1p[:NH] = b1
    b1c = np.ascontiguousarray(b1p.reshape(NCHUNK, 128).T)          # [128, 8]
    W2e = np.zeros((HPAD, NO), f32)
    W2e[:NH] = 0.5 * W2.astype(f32, copy=False).T
    w2stack = np.ascontiguousarray(W2e.reshape(NCHUNK, 128, NO).transpose(1, 0, 2))
    w2hi = w2stack.astype(ml_dtypes.bfloat16)
    w2lo = (w2stack - w2hi.astype(f32)).astype(ml_dtypes.bfloat16)
    b2eff = (b2.astype(f32) + 0.5 * W2.astype(f32).sum(axis=1)).reshape(NO, 1)
    b2eff = np.ascontiguousarray(b2eff.astype(f32))

    in_maps = []
    for cidx in range(N_CORES):
        ksl = slice(cidx * KC, (cidx + 1) * KC)
        xh = xTh[ksl].reshape(KTILES, 128, B).transpose(1, 0, 2)
        xl = xTl[ksl].reshape(KTILES, 128, B).transpose(1, 0, 2)
        # [KC, HPAD] -> [KTILES, 128, 4, 256] -> [4, 128, KTILES, 256]
        wh = w1Th[ksl].reshape(KTILES, 128, 4, 256).transpose(2, 1, 0, 3)
        wl = w1Tl[ksl].reshape(KTILES, 128, 4, 256).transpose(2, 1, 0, 3)
        in_maps.append({
            "xth": np.ascontiguousarray(xh),
            "xtl": np.ascontiguousarray(xl),
            "w1h": np.ascontiguousarray(wh),
            "w1l": np.ascontiguousarray(wl),
            "b1c": b1c,
            "w2hi": w2hi,
            "w2lo": w2lo,
            "b2c": b2eff,
        })
    return in_maps


def _gather(results):
    spk_parts, mem_parts = [], []
    for r in results:
        mem_parts.append(r["mem2rec"].transpose(1, 2, 0))  # [T, BLOC, NO]
        spk_parts.append(r["spk2rec"].transpose(1, 2, 0))
    mem2 = np.concatenate(mem_parts, axis=1).astype(np.float32)  # [200, 256, 5]
    spk2 = np.concatenate(spk_parts, axis=1).astype(np.float32)
    return spk2, mem2


def run_raw(inputs, dbg=False, **kwargs):
    """Build+run; returns BassKernelResults (for profiling from test.py)."""
    from concourse.bass_utils import run_bass_kernel_spmd

    nc, _, _ = _build_program(dbg=dbg)
    in_maps = _prep_inputs(**inputs)
    return run_bass_kernel_spmd(nc, in_maps, core_ids=list(range(N_CORES)), **kwargs)


def kernel(x, W1, b1, W2, b2):
    res = run_raw(dict(x=x, W1=W1, b1=b1, W2=W2, b2=b2))
    return _gather(res.results)


if __name__ == "__main__":
    rng = np.random.default_rng(0)
    ins = {
        "x": rng.standard_normal((B, 2, 80, 200)).astype(np.float32),
        "W1": rng.uniform(-1, 1, (NH, NIN)).astype(np.float32) / np.sqrt(NIN),
        "b1": rng.uniform(-1, 1, NH).astype(np.float32) / np.sqrt(NIN),
        "W2": rng.uniform(-1, 1, (NO, NH)).astype(np.float32) / np.sqrt(NH),
        "b2": rng.uniform(-1, 1, NO).astype(np.float32) / np.sqrt(NH),
    }
    spk2, mem2 = kernel(**ins)
    print("shapes:", spk2.shape, mem2.shape, spk2.dtype, mem2.dtype)
    print("spk2 mean:", spk2.mean(), "mem2 std:", mem2.std())


# revision 34
# speedup vs baseline: 1.2634x; 1.0375x over previous
"""Trainium2 Bass kernel for the SNN (LIF) network:

    cur1 = x.reshape(B,-1) @ W1.T + b1          (big fp32 matmul, once)
    200 sequential LIF steps on [B,1000] (layer 1), tiny matmul into 5
    outputs per step (layer 2), second LIF on [B,5].

Distribution over 8 cores (v3):
  Phase A: contraction(K)-sharded exact-fp32 matmul (fp16 hi/lo, 3
           passes), looped hidden-chunk (128 cols) OUTER / K-tile inner
           with x resident in SBUF. Each chunk's partial [256, 128]
           goes through its own ReduceScatter(add) immediately, so all
           8 collectives overlap the remaining chunks' matmuls; each is
           followed by a pipelined PE-transpose + b1 bias into the scan
           layout. Each core ends with curb [128h, (chunk, 32batch)].
  Phase B: per-core LIF layer-1 scan, hidden on partitions. One custom
           DVE instruction per step into a per-group (G=4) buffer;
           ONE batched ACT Sign per group converts the whole group's
           mems to g=sign(m-1) in fp16.
  Phase C: per group, W2 chunks (bf16 hi+lo) are the PE *stationary*
           operands (LDW is 5 cols = ~4ns) and the spikes stream as the
           moving operand; 16 accumulating matmuls -> PSUM [5, 4, 32].
           ACT drains each group into c2stage [5, T, 32] (no bias).
  Phase D: layer-2 LIF runs 2 groups behind as custom DVE ops directly
           in [5, 32] layout; the op folds the effective bias b2eff in
           via the per-partition C1 scalar. mem2rec [5, T, 32] is both
           the recurrent state chain and the recorded output. GPSIMD
           extracts spk2 in 8-group batches; outputs DMA out in blocks.
"""
import os
import sys

if "/opt/trn_rl_repo" not in sys.path:
    sys.path.insert(0, "/opt/trn_rl_repo")

# Profile every core when NTFF tracing is on: exec time = max per-core span
# with aligned starts, instead of core 0's span inflated by the runtime's
# per-device dispatch stagger while it waits at the collective.
os.environ.setdefault("BASS_PERFETTO_PROFILE_ALL_CORES", "1")

import numpy as np
import ml_dtypes

# ---------------------------------------------------------------- constants
BETA = 0.95
T = 200
B = 256
NIN = 32000
NH = 1000
NO = 5

N_CORES = 8
KPAD = 32768           # NIN padded to 256*128
KC = KPAD // N_CORES   # 4096 contraction per core
KTILES = KC // 128     # 32
HPAD = 1024            # hidden padded
BLOC = B // N_CORES    # 32 batch rows per core after ReduceScatter
NCHUNK = HPAD // 128   # 8 hidden chunks of 128
G = 4                  # group size (steps per PE batch)
NGROUP = T // G        # 50
DLAG = 2               # layer-2 group lag behind layer-1
OBLK = 8               # groups per spk2/output batch
W1SCALE = 256.0        # W1 pre-scale so the fp16 lo-half stays normal

# ---------------------------------------------------------------- custom ops
_LIF_NAME = "LIF_STEP_ANT"
_LIF2_NAME = "LIF2B_STEP_ANT"


def _register_lif_ops():
    from concourse.dve_ops import (
        DveOp, OPS, CUSTOM_DVE_SPECS, _SUB_OPCODE_FOR_NAME, _CUSTOM_DVE_ROW_BASE,
    )
    from concourse.dve_spec import Spec, Src0, Src1, C0, C1, One, lower as dve_lower, _has_src1
    from concourse.dve_uop import DveOpSpec

    def _mk(name, spec):
        for op in OPS:
            if op.name == name:
                return op
        if name not in _SUB_OPCODE_FOR_NAME:
            _SUB_OPCODE_FOR_NAME[name] = _CUSTOM_DVE_ROW_BASE + len(OPS)
        shas = {}
        for ver in ("v3", "v4"):
            s = DveOpSpec(
                name=name,
                opcode=_SUB_OPCODE_FOR_NAME[name],
                uops=dve_lower(spec, ver=ver),
                rd1_en=_has_src1(spec),
            )
            shas[ver] = s.sha(ver)
        op = DveOp(name, spec, subdim=False, uops_sha=shas)
        OPS.append(op)
        CUSTOM_DVE_SPECS[name] = op.spec
        return op

    lif = _mk(_LIF_NAME, Spec(
        body=Src0 * C0 + Src1 - (Src0 > One),
        reference=lambda in0, in1, s0: in0 * s0 + in1 - (in0 > 1.0).astype(np.float32),
    ))
    lif2 = _mk(_LIF2_NAME, Spec(
        body=Src0 * C0 + Src1 + C1 - (Src0 > One),
        reference=lambda in0, in1, s0, s1:
            in0 * s0 + in1 + s1 - (in0 > 1.0).astype(np.float32),
    ))
    return lif, lif2


# ---------------------------------------------------------------- program
_PROGRAMS = {}


def _build_program(sim=False, dbg=False):
    key = (sim, dbg)
    if key in _PROGRAMS:
        return _PROGRAMS[key]

    import concourse.bass as bass
    import concourse.tile as tile
    from concourse import bacc, mybir
    from concourse.masks import make_identity

    LIF, LIF2 = _register_lif_ops()
    f32 = mybir.dt.float32
    bf16 = mybir.dt.bfloat16
    f16 = mybir.dt.float16

    nc = bacc.Bacc("TRN2", target_bir_lowering=False, debug=False,
                   num_devices=1 if sim else N_CORES)

    # inputs (per-core)
    xth_d = nc.dram_tensor("xth", [128, KTILES, B], f16, kind="ExternalInput").ap()
    xtl_d = nc.dram_tensor("xtl", [128, KTILES, B], f16, kind="ExternalInput").ap()
    # quarter-major W1: [quarter, K-within-tile partition, KTILES, 256 hidden cols]
    w1h_d = nc.dram_tensor("w1h", [4, 128, KTILES, 256], f16, kind="ExternalInput").ap()
    w1l_d = nc.dram_tensor("w1l", [4, 128, KTILES, 256], f16, kind="ExternalInput").ap()
    b1c_d = nc.dram_tensor("b1c", [128, NCHUNK], f32, kind="ExternalInput").ap()
    w2hi_d = nc.dram_tensor("w2hi", [128, NCHUNK, NO], bf16, kind="ExternalInput").ap()
    w2lo_d = nc.dram_tensor("w2lo", [128, NCHUNK, NO], bf16, kind="ExternalInput").ap()
    b2c_d = nc.dram_tensor("b2c", [NO, 1], f32, kind="ExternalInput").ap()
    # outputs (per-core batch slice), layout (o, t, b)
    mem2_d = nc.dram_tensor("mem2rec", [NO, T, BLOC], f32, kind="ExternalOutput").ap()
    spk2_d = nc.dram_tensor("spk2rec", [NO, T, BLOC], f32, kind="ExternalOutput").ap()
    if dbg:
        curdbg_d = nc.dram_tensor("curdbg", [128, NCHUNK * BLOC], f32,
                                  kind="ExternalOutput").ap()
        c2dbg_d = nc.dram_tensor("c2dbg", [NO, T, BLOC], f32,
                                 kind="ExternalOutput").ap()

    with tile.TileContext(nc) as tc:
        with (
            tc.tile_pool(name="xres", bufs=1) as xres,
            tc.tile_pool(name="win", bufs=2) as wpool,
            tc.tile_pool(name="psA", bufs=2, space="PSUM") as psA,
            tc.tile_pool(name="cs", bufs=2) as cspool,
            tc.tile_pool(name="rsb", bufs=2) as rsbpool,
            tc.tile_pool(name="psT", bufs=2, space="PSUM") as psT,
            tc.tile_pool(name="stage", bufs=1) as stage,
            tc.tile_pool(name="dram", bufs=1, space="DRAM") as dram,
            tc.tile_pool(name="mg", bufs=3) as mpool,
            tc.tile_pool(name="mgA", bufs=3) as mpoolA,
            tc.tile_pool(name="mgB", bufs=3) as mpoolB,
            tc.tile_pool(name="gt", bufs=10) as gpool,
            tc.tile_pool(name="psC", bufs=2, space="PSUM") as psC,
            tc.tile_pool(name="sblk", bufs=2) as spool,
            tc.tile_pool(name="c2g", bufs=10) as cpool,
        ):
            # ---------------- phase A: chunk-outer K-sharded matmul
            xall_h = xres.tile([128, KTILES, B], f16, tag="xah")
            xall_l = xres.tile([128, KTILES, B], f16, tag="xal")

            # persistent small constants, loaded early
            b1t = stage.tile([128, NCHUNK], f32, tag="b1t")
            nc.sync.dma_start(b1t[:], b1c_d[:])
            ident = stage.tile([BLOC, BLOC], f32, tag="ident")
            make_identity(nc, ident[:])

            curb = stage.tile([128, NCHUNK * BLOC], f32, tag="curb")

            # hidden quarters (2 chunks each): shared x stationaries across
            # the quarter's 2 chunks keep LDWEIGHTS (2 per 6 matmuls) hidden,
            # while each quarter's ReduceScatter overlaps the next quarter's
            # matmuls.
            rs_qs = []
            for qd in range(4):
                wh = wpool.tile([128, KTILES, 256], f16, tag="wh")
                wl = wpool.tile([128, KTILES, 256], f16, tag="wl")
                if qd == 0:
                    # stage a minimal first x piece + first weight chunk so
                    # the PE can start as early as possible.
                    nc.sync.dma_start(xall_h[:, 0:4, :], xth_d[:, 0:4, :])
                    nc.sync.dma_start(xall_l[:, 0:4, :], xtl_d[:, 0:4, :])
                    nc.sync.dma_start(wh[:, 0:4, :], w1h_d[qd][:, 0:4, :])
                    nc.sync.dma_start(wl[:, 0:4, :], w1l_d[qd][:, 0:4, :])
                    nc.sync.dma_start(xall_h[:, 4:12, :], xth_d[:, 4:12, :])
                    nc.sync.dma_start(xall_l[:, 4:12, :], xtl_d[:, 4:12, :])
                    nc.sync.dma_start(wh[:, 4:12, :], w1h_d[qd][:, 4:12, :])
                    nc.sync.dma_start(wl[:, 4:12, :], w1l_d[qd][:, 4:12, :])
                    for q in range(3):
                        ktq = slice(12 + q * 8, min(12 + (q + 1) * 8, 32))
                        nc.sync.dma_start(xall_h[:, ktq, :], xth_d[:, ktq, :])
                        nc.sync.dma_start(xall_l[:, ktq, :], xtl_d[:, ktq, :])
                        nc.sync.dma_start(wh[:, ktq, :], w1h_d[qd][:, ktq, :])
                        nc.sync.dma_start(wl[:, ktq, :], w1l_d[qd][:, ktq, :])
                else:
                    # chunked weight DMAs: matmuls for kt-range r start as
                    # soon as chunk r lands, and the WAR release on the
                    # rotated buffer is finer-grained.
                    for r in range(4):
                        ktr = slice(r * 8, (r + 1) * 8)
                        nc.sync.dma_start(wh[:, ktr, :], w1h_d[qd][:, ktr, :])
                        nc.sync.dma_start(wl[:, ktr, :], w1l_d[qd][:, ktr, :])

                # both mb accumulators packed into one PSUM bank-tile
                psq = psA.tile([128, 2, 256], f32, tag="psq")
                ps = [psq[:, mb, :] for mb in range(2)]
                for kt in range(KTILES):
                    for mb in range(2):
                        xh_s = xall_h[:, kt, mb * 128:(mb + 1) * 128]
                        xl_s = xall_l[:, kt, mb * 128:(mb + 1) * 128]
                        # start=True clears has_written for the WHOLE bank, so
                        # only the very first matmul into the packed bank-tile
                        # may carry it; the other region's first write lands on
                        # cleared bits and overwrites per the per-element rule.
                        nc.tensor.matmul(ps[mb], xh_s, wl[:, kt, :],
                                         start=(kt == 0 and mb == 0),
                                         stop=False)
                        nc.tensor.matmul(ps[mb], xh_s, wh[:, kt, :],
                                         start=False, stop=False)
                        nc.tensor.matmul(ps[mb], xl_s, wh[:, kt, :],
                                         start=False, stop=(kt == KTILES - 1))

                # drain (scale back) + store partial to DRAM, batch-major
                partial = dram.tile([B, 256], f32, tag=f"pa{qd}", name=f"partial{qd}")
                for mb in range(2):
                    csb = cspool.tile([128, 256], f32, tag=f"cs{mb}",
                                      name=f"cs{mb}_{qd}")
                    nc.scalar.activation(csb[:], ps[mb],
                                         mybir.ActivationFunctionType.Copy,
                                         scale=1.0 / W1SCALE)
                    # scalar queue: keeps the sync queue a pure input-prefetch
                    # stream (in-order queues head-of-line block otherwise)
                    nc.scalar.dma_start(partial[mb * 128:(mb + 1) * 128, :], csb[:])

                rs_q = dram.tile([BLOC, 256], f32, tag=f"rs{qd}", name=f"rs{qd}")
                if sim:
                    nc.sync.dma_start(rs_q[:], partial[0:BLOC, :])
                else:
                    nc.gpsimd.collective_compute(
                        "ReduceScatter",
                        mybir.AluOpType.add,
                        replica_groups=[list(range(N_CORES))],
                        ins=[partial.opt()],
                        outs=[rs_q.opt()],
                    )
                rs_qs.append(rs_q)

            # RS-dependent work LAST, so nothing that waits on a collective
            # sits ahead of later quarters' matmuls in any in-order queue.
            # rsb loads ride the scalar queue after all partial stores;
            # b1 folds in via a DVE tensor_scalar with a [P,1] scalar AP.
            for qd in range(4):
                rsb = rsbpool.tile([BLOC, 256], f32, tag="rsb",
                                   name=f"rsb{qd}")
                nc.scalar.dma_start(rsb[:], rs_qs[qd][:])
                for cl in range(2):
                    c = 2 * qd + cl
                    pt = psT.tile([128, BLOC], f32, tag="pst",
                                  name=f"pst{qd}{cl}")
                    nc.tensor.matmul(pt[:], rsb[:, cl * 128:(cl + 1) * 128],
                                     ident[:], start=True, stop=True,
                                     is_transpose=True)
                    nc.vector.tensor_scalar(
                        curb[:, c * BLOC:(c + 1) * BLOC], pt[:],
                        b1t[:, c:c + 1], None, mybir.AluOpType.add)

            if dbg:
                nc.sync.dma_start(curdbg_d[:], curb[:])

            # ---------------- scan constants
            w2hi_t = stage.tile([128, NCHUNK, NO], bf16, tag="w2hi")
            nc.sync.dma_start(w2hi_t[:], w2hi_d[:])
            w2lo_t = stage.tile([128, NCHUNK, NO], bf16, tag="w2lo")
            nc.sync.dma_start(w2lo_t[:], w2lo_d[:])
            b2c_t = stage.tile([NO, 1], f32, tag="b2c")
            nc.sync.dma_start(b2c_t[:], b2c_d[:])
            biasm1 = stage.tile([128, 1], f32, tag="bm1")
            nc.vector.memset(biasm1[:], -1.0)
            zeros_t = stage.tile([128, NCHUNK * BLOC], f32, tag="zeros")
            nc.vector.memset(zeros_t[:], 0.0)
            z5 = stage.tile([NO, BLOC], f32, tag="z5")
            nc.vector.memset(z5[:], 0.0)
            mem2rec = stage.tile([NO, T, BLOC], f32, tag="m2r")
            c2tiles = [None] * NGROUP  # rotating per-group cur2 tiles

            def d_step(dt):
                """Layer-2 LIF step dt (0-based) on DVE, [5, 32] layout.

                b2eff is pre-folded into c2 by the drain's ACT bias, so the
                plain 1-scalar LIF op suffices (the [P,1] C1 variant costs
                ~+60ns/op in AP latch setup).
                """
                in0 = z5[:] if dt == 0 else mem2rec[:, dt - 1, :]
                nc.vector._custom_dve(
                    LIF,
                    out=mem2rec[:, dt, :],
                    in0=in0,
                    in1=c2tiles[dt // G][:, dt % G, :],
                    s0=BETA,
                )

            def out_block(t0, t1):
                # DVE, not GPSIMD: a [5, N] gpsimd tensor_scalar measures
                # ~13.7us and stalls concurrent DVE work via the shared SBUF
                # port; the DVE op is ~0.7us.
                sblk = spool.tile([NO, (OBLK + DLAG) * G, BLOC], f32, tag="sblk")
                nc.vector.tensor_scalar(
                    sblk[:, 0:t1 - t0, :], mem2rec[:, t0:t1, :], 1.0, None,
                    mybir.AluOpType.is_gt)
                nc.sync.dma_start(mem2_d[:, t0:t1, :], mem2rec[:, t0:t1, :])
                nc.sync.dma_start(spk2_d[:, t0:t1, :], sblk[:, 0:t1 - t0, :])

            # ---------------- phase B/C/D: fused scan
            # The first SPLIT_G groups run the layer-1 LIF column-split:
            # chunks 0-5 (quarters 1-3 of curb, ready first) scan ahead on
            # DVE while quarter 4's ReduceScatter is still in flight; chunks
            # 6-7 catch up afterwards. Group-end work (PE contraction, drain,
            # layer-2) for those groups is deferred until both halves exist.
            SPLIT_G = 8
            ASZ = 6 * BLOC   # chunks 0-5
            BSZ = 2 * BLOC   # chunks 6-7

            def group_tail(gi, gt):
                """PE contraction + cur2 drain for group gi."""
                pc = psC.tile([NO, G, BLOC], f32, tag="pc", name=f"pc{gi}")
                for c2 in range(NCHUNK):
                    rhs = gt[:, :, c2, :]
                    nc.tensor.matmul(pc[:], w2hi_t[:, c2, :], rhs,
                                     start=(c2 == 0), stop=False)
                    nc.tensor.matmul(pc[:], w2lo_t[:, c2, :], rhs,
                                     start=False, stop=(c2 == NCHUNK - 1))
                # drain, folding b2eff in via the per-partition ACT bias
                c2g = cpool.tile([NO, G, BLOC], f32, tag="c2g", name=f"c2g{gi}")
                c2tiles[gi] = c2g
                nc.scalar.activation(
                    c2g[:], pc[:], mybir.ActivationFunctionType.Identity,
                    bias=b2c_t[:], scale=1.0)
                if dbg:
                    nc.sync.dma_start(c2dbg_d[:, gi * G:(gi + 1) * G, :],
                                      c2g[:])

            # phase 1: A-columns scan-ahead for the first SPLIT_G groups
            memA = zeros_t[:, 0:ASZ]
            gts = [None] * SPLIT_G
            mgAs = [None] * SPLIT_G
            for t in range(1, SPLIT_G * G + 1):
                gi, sl = (t - 1) // G, (t - 1) % G
                if sl == 0:
                    mgAs[gi] = mpoolA.tile([128, G, ASZ], f32, tag="mgA",
                                           name=f"mgA{gi}")
                out_v = mgAs[gi][:, sl, :]
                nc.vector._custom_dve(LIF, out=out_v, in0=memA,
                                      in1=curb[:, 0:ASZ], s0=BETA)
                memA = out_v
                if sl == G - 1:
                    gts[gi] = gpool.tile([128, G, NCHUNK, BLOC], bf16,
                                         tag="gt", name=f"gt{gi}")
                    nc.scalar.activation(
                        gts[gi][:, :, 0:6, :],
                        mgAs[gi][:].rearrange("p g (c b) -> p g c b", b=BLOC),
                        mybir.ActivationFunctionType.Sign,
                        bias=biasm1[:], scale=1.0,
                    )

            # phase 2 prologue: B-columns catch up; full group tails follow
            memB = zeros_t[:, ASZ:]
            mgB = None
            for t in range(1, SPLIT_G * G + 1):
                gi, sl = (t - 1) // G, (t - 1) % G
                if sl == 0:
                    mgB = mpoolB.tile([128, G, BSZ], f32, tag="mgB")
                out_v = mgB[:, sl, :]
                nc.vector._custom_dve(LIF, out=out_v, in0=memB,
                                      in1=curb[:, ASZ:], s0=BETA)
                memB = out_v
                if sl == G - 1:
                    nc.scalar.activation(
                        gts[gi][:, :, 6:8, :],
                        mgB[:].rearrange("p g (c b) -> p g c b", b=BLOC),
                        mybir.ActivationFunctionType.Sign,
                        bias=biasm1[:], scale=1.0,
                    )
                    group_tail(gi, gts[gi])
                    gts[gi] = None

            # merge the split state into a unified tile for step SPLIT_G*G
            mg = mpool.tile([128, G, NCHUNK * BLOC], f32, tag="mg")
            nc.vector.tensor_copy(mg[:, G - 1, 0:ASZ],
                                  mgAs[SPLIT_G - 1][:, G - 1, :])
            nc.vector.tensor_copy(mg[:, G - 1, ASZ:], memB)
            mem_prev = mg[:, G - 1, :]

            out_done = 0
            d_done = 0
            for t in range(SPLIT_G * G + 1, T + 1):
                gi, sl = (t - 1) // G, (t - 1) % G
                if sl == 0:
                    mg = mpool.tile([128, G, NCHUNK * BLOC], f32, tag="mg")
                out_v = mg[:, sl, :]
                nc.vector._custom_dve(LIF, out=out_v, in0=mem_prev,
                                      in1=curb[:], s0=BETA)
                mem_prev = out_v
                if sl == G - 1:
                    # batched sign for the whole group -> g in {-1,+1} bf16
                    gt = gpool.tile([128, G, NCHUNK, BLOC], bf16, tag="gt",
                                    name=f"gt{gi}")
                    nc.scalar.activation(
                        gt[:].rearrange("p g c b -> p (g c b)"),
                        mg[:].rearrange("p g f -> p (g f)"),
                        mybir.ActivationFunctionType.Sign,
                        bias=biasm1[:], scale=1.0,
                    )
                    group_tail(gi, gt)
                    # layer-2 LIF, lagged DLAG groups (flushes the deferred
                    # split-phase groups on the first unified group)
                    if gi >= DLAG:
                        for dg in range(d_done, gi - DLAG + 1):
                            for dt in range(dg * G, (dg + 1) * G):
                                d_step(dt)
                        d_done = gi - DLAG + 1
                    # spk2 + output DMA in OBLK-group blocks, lagged
                    if gi >= DLAG + OBLK and (gi - DLAG - OBLK) % OBLK == 0:
                        blk = (gi - DLAG - OBLK) // OBLK
                        out_block(blk * OBLK * G, (blk + 1) * OBLK * G)
                        out_done = (blk + 1) * OBLK * G
                    if gi == NGROUP - 1:
                        # flush everything already d-stepped (t < (gi-1)*G)
                        out_block(out_done, (gi - DLAG + 1) * G)
                        out_done = (gi - DLAG + 1) * G

            # ---------------- tail
            for dt in range((NGROUP - DLAG) * G, T):
                d_step(dt)
            out_block(out_done, T)

    nc.compile()
    _PROGRAMS[key] = (nc, LIF, LIF2)
    return _PROGRAMS[key]


# ---------------------------------------------------------------- host prep
def _prep_inputs(x, W1, b1, W2, b2):
    f32 = np.float32
    f16 = np.float16
    x_flat = np.ascontiguousarray(x.reshape(B, -1).astype(f32, copy=False))
    xT = np.zeros((KPAD, B), f32)
    xT[:NIN] = x_flat.T
    xTh = xT.astype(f16)
    xTl = (xT - xTh.astype(f32)).astype(f16)
    w1T = np.zeros((KPAD, HPAD), f32)
    w1T[:NIN, :NH] = W1.astype(f32, copy=False).T * W1SCALE
    w1Th = w1T.astype(f16)
    w1Tl = (w1T - w1Th.astype(f32)).astype(f16)
    b1p = np.full(HPAD, -10.0, f32)
    b1p[:NH] = b1
    b1c = np.ascontiguousarray(b1p.reshape(NCHUNK, 128).T)          # [128, 8]
    W2e = np.zeros((HPAD, NO), f32)
    W2e[:NH] = 0.5 * W2.astype(f32, copy=False).T
    w2stack = np.ascontiguousarray(W2e.reshape(NCHUNK, 128, NO).transpose(1, 0, 2))
    w2hi = w2stack.astype(ml_dtypes.bfloat16)
    w2lo = (w2stack - w2hi.astype(f32)).astype(ml_dtypes.bfloat16)
    b2eff = (b2.astype(f32) + 0.5 * W2.astype(f32).sum(axis=1)).reshape(NO, 1)
    b2eff = np.ascontiguousarray(b2eff.astype(f32))

    in_maps = []
    for cidx in range(N_CORES):
        ksl = slice(cidx * KC, (cidx + 1) * KC)
        xh = xTh[ksl].reshape(KTILES, 128, B).transpose(1, 0, 2)
        xl = xTl[ksl].reshape(KTILES, 128, B).transpose(1, 0, 2)
        # [KC, HPAD] -> [KTILES, 128, 4, 256] -> [4, 128, KTILES, 256]
        wh = w1Th[ksl].reshape(KTILES, 128, 4, 256).transpose(2, 1, 0, 3)
        wl = w1Tl[ksl].reshape(KTILES, 128, 4, 256).transpose(2, 1, 0, 3)
        in_maps.append({
            "xth": np.ascontiguousarray(xh),
            "xtl": np.ascontiguousarray(xl),
            "w1h": np.ascontiguousarray(wh),
            "w1l": np.ascontiguousarray(wl),
            "b1c": b1c,
            "w2hi": w2hi,
            "w2lo": w2lo,
            "b2c": b2eff,
        })
    return in_maps


def _gather(results):
    spk_parts, mem_parts = [], []
    for r in results:
        mem_parts.append(r["mem2rec"].transpose(1, 2, 0))  # [T, BLOC, NO]
        spk_parts.append(r["spk2rec"].transpose(1, 2, 0))
    mem2 = np.concatenate(mem_parts, axis=1).astype(np.float32)  # [200, 256, 5]
    spk2 = np.concatenate(spk_parts, axis=1).astype(np.float32)
    return spk2, mem2


def run_raw(inputs, dbg=False, **kwargs):
    """Build+run; returns BassKernelResults (for profiling from test.py)."""
    from concourse.bass_utils import run_bass_kernel_spmd

    nc, _, _ = _build_program(dbg=dbg)
    in_maps = _prep_inputs(**inputs)
    return run_bass_kernel_spmd(nc, in_maps, core_ids=list(range(N_CORES)), **kwargs)


def kernel(x, W1, b1, W2, b2):
    res = run_raw(dict(x=x, W1=W1, b1=b1, W2=W2, b2=b2))
    return _gather(res.results)


if __name__ == "__main__":
    rng = np.random.default_rng(0)
    ins = {
        "x": rng.standard_normal((B, 2, 80, 200)).astype(np.float32),
        "W1": rng.uniform(-1, 1, (NH, NIN)).astype(np.float32) / np.sqrt(NIN),
        "b1": rng.uniform(-1, 1, NH).astype(np.float32) / np.sqrt(NIN),
        "W2": rng.uniform(-1, 1, (NO, NH)).astype(np.float32) / np.sqrt(NH),
        "b2": rng.uniform(-1, 1, NO).astype(np.float32) / np.sqrt(NH),
    }
    spk2, mem2 = kernel(**ins)
    print("shapes:", spk2.shape, mem2.shape, spk2.dtype, mem2.dtype)
    print("spk2 mean:", spk2.mean(), "mem2 std:", mem2.std())


# revision 36
# speedup vs baseline: 1.3032x; 1.0315x over previous
"""Trainium2 Bass kernel for the SNN (LIF) network:

    cur1 = x.reshape(B,-1) @ W1.T + b1          (big fp32 matmul, once)
    200 sequential LIF steps on [B,1000] (layer 1), tiny matmul into 5
    outputs per step (layer 2), second LIF on [B,5].

Distribution over 8 cores (v3):
  Phase A: contraction(K)-sharded exact-fp32 matmul (fp16 hi/lo, 3
           passes), looped hidden-chunk (128 cols) OUTER / K-tile inner
           with x resident in SBUF. Each chunk's partial [256, 128]
           goes through its own ReduceScatter(add) immediately, so all
           8 collectives overlap the remaining chunks' matmuls; each is
           followed by a pipelined PE-transpose + b1 bias into the scan
           layout. Each core ends with curb [128h, (chunk, 32batch)].
  Phase B: per-core LIF layer-1 scan, hidden on partitions. One custom
           DVE instruction per step into a per-group (G=4) buffer;
           ONE batched ACT Sign per group converts the whole group's
           mems to g=sign(m-1) in fp16.
  Phase C: per group, W2 chunks (bf16 hi+lo) are the PE *stationary*
           operands (LDW is 5 cols = ~4ns) and the spikes stream as the
           moving operand; 16 accumulating matmuls -> PSUM [5, 4, 32].
           ACT drains each group into c2stage [5, T, 32] (no bias).
  Phase D: layer-2 LIF runs 2 groups behind as custom DVE ops directly
           in [5, 32] layout; the op folds the effective bias b2eff in
           via the per-partition C1 scalar. mem2rec [5, T, 32] is both
           the recurrent state chain and the recorded output. GPSIMD
           extracts spk2 in 8-group batches; outputs DMA out in blocks.
"""
import os
import sys

if "/opt/trn_rl_repo" not in sys.path:
    sys.path.insert(0, "/opt/trn_rl_repo")

# Profile every core when NTFF tracing is on: exec time = max per-core span
# with aligned starts, instead of core 0's span inflated by the runtime's
# per-device dispatch stagger while it waits at the collective.
os.environ.setdefault("BASS_PERFETTO_PROFILE_ALL_CORES", "1")

import numpy as np
import ml_dtypes

# ---------------------------------------------------------------- constants
BETA = 0.95
T = 200
B = 256
NIN = 32000
NH = 1000
NO = 5

N_CORES = 8
KPAD = 32768           # NIN padded to 256*128
KC = KPAD // N_CORES   # 4096 contraction per core
KTILES = KC // 128     # 32
HPAD = 1024            # hidden padded
BLOC = B // N_CORES    # 32 batch rows per core after ReduceScatter
NCHUNK = HPAD // 128   # 8 hidden chunks of 128
G = 4                  # group size (steps per PE batch)
NGROUP = T // G        # 50
DLAG = 2               # layer-2 group lag behind layer-1
OBLK = 8               # groups per spk2/output batch
W1SCALE = 256.0        # W1 pre-scale so the fp16 lo-half stays normal

# ---------------------------------------------------------------- custom ops
_LIF_NAME = "LIF_STEP_ANT"
_LIF2_NAME = "LIF2B_STEP_ANT"


def _register_lif_ops():
    from concourse.dve_ops import (
        DveOp, OPS, CUSTOM_DVE_SPECS, _SUB_OPCODE_FOR_NAME, _CUSTOM_DVE_ROW_BASE,
    )
    from concourse.dve_spec import Spec, Src0, Src1, C0, C1, One, lower as dve_lower, _has_src1
    from concourse.dve_uop import DveOpSpec

    def _mk(name, spec):
        for op in OPS:
            if op.name == name:
                return op
        if name not in _SUB_OPCODE_FOR_NAME:
            _SUB_OPCODE_FOR_NAME[name] = _CUSTOM_DVE_ROW_BASE + len(OPS)
        shas = {}
        for ver in ("v3", "v4"):
            s = DveOpSpec(
                name=name,
                opcode=_SUB_OPCODE_FOR_NAME[name],
                uops=dve_lower(spec, ver=ver),
                rd1_en=_has_src1(spec),
            )
            shas[ver] = s.sha(ver)
        op = DveOp(name, spec, subdim=False, uops_sha=shas)
        OPS.append(op)
        CUSTOM_DVE_SPECS[name] = op.spec
        return op

    lif = _mk(_LIF_NAME, Spec(
        body=Src0 * C0 + Src1 - (Src0 > One),
        reference=lambda in0, in1, s0: in0 * s0 + in1 - (in0 > 1.0).astype(np.float32),
    ))
    lif2 = _mk(_LIF2_NAME, Spec(
        body=Src0 * C0 + Src1 + C1 - (Src0 > One),
        reference=lambda in0, in1, s0, s1:
            in0 * s0 + in1 + s1 - (in0 > 1.0).astype(np.float32),
    ))
    return lif, lif2


# ---------------------------------------------------------------- program
_PROGRAMS = {}


def _build_program(sim=False, dbg=False):
    key = (sim, dbg)
    if key in _PROGRAMS:
        return _PROGRAMS[key]

    import concourse.bass as bass
    import concourse.tile as tile
    from concourse import bacc, mybir
    from concourse.masks import make_identity

    LIF, LIF2 = _register_lif_ops()
    f32 = mybir.dt.float32
    bf16 = mybir.dt.bfloat16
    f16 = mybir.dt.float16

    nc = bacc.Bacc("TRN2", target_bir_lowering=False, debug=False,
                   num_devices=1 if sim else N_CORES)

    # inputs (per-core)
    xth_d = nc.dram_tensor("xth", [128, KTILES, B], f16, kind="ExternalInput").ap()
    xtl_d = nc.dram_tensor("xtl", [128, KTILES, B], f16, kind="ExternalInput").ap()
    # quarter-major W1: [quarter, K-within-tile partition, KTILES, 256 hidden cols]
    w1h_d = nc.dram_tensor("w1h", [4, 128, KTILES, 256], f16, kind="ExternalInput").ap()
    w1l_d = nc.dram_tensor("w1l", [4, 128, KTILES, 256], f16, kind="ExternalInput").ap()
    b1c_d = nc.dram_tensor("b1c", [128, NCHUNK], f32, kind="ExternalInput").ap()
    w2hi_d = nc.dram_tensor("w2hi", [128, NCHUNK, NO], bf16, kind="ExternalInput").ap()
    w2lo_d = nc.dram_tensor("w2lo", [128, NCHUNK, NO], bf16, kind="ExternalInput").ap()
    b2c_d = nc.dram_tensor("b2c", [NO, 1], f32, kind="ExternalInput").ap()
    # outputs (per-core batch slice), layout (o, t, b)
    mem2_d = nc.dram_tensor("mem2rec", [NO, T, BLOC], f32, kind="ExternalOutput").ap()
    spk2_d = nc.dram_tensor("spk2rec", [NO, T, BLOC], f32, kind="ExternalOutput").ap()
    if dbg:
        curdbg_d = nc.dram_tensor("curdbg", [128, NCHUNK * BLOC], f32,
                                  kind="ExternalOutput").ap()
        c2dbg_d = nc.dram_tensor("c2dbg", [NO, T, BLOC], f32,
                                 kind="ExternalOutput").ap()

    with tile.TileContext(nc) as tc:
        with (
            tc.tile_pool(name="xres", bufs=1) as xres,
            tc.tile_pool(name="win", bufs=2) as wpool,
            tc.tile_pool(name="psA", bufs=2, space="PSUM") as psA,
            tc.tile_pool(name="cs", bufs=2) as cspool,
            tc.tile_pool(name="rsb", bufs=2) as rsbpool,
            tc.tile_pool(name="psT", bufs=2, space="PSUM") as psT,
            tc.tile_pool(name="stage", bufs=1) as stage,
            tc.tile_pool(name="dram", bufs=1, space="DRAM") as dram,
            tc.tile_pool(name="mg", bufs=3) as mpool,
            tc.tile_pool(name="mgA", bufs=3) as mpoolA,
            tc.tile_pool(name="mgB", bufs=3) as mpoolB,
            tc.tile_pool(name="gt", bufs=14) as gpool,
            tc.tile_pool(name="psC", bufs=2, space="PSUM") as psC,
            tc.tile_pool(name="sblk", bufs=2) as spool,
            tc.tile_pool(name="c2g", bufs=14) as cpool,
        ):
            # ---------------- phase A: chunk-outer K-sharded matmul
            xall_h = xres.tile([128, KTILES, B], f16, tag="xah")
            xall_l = xres.tile([128, KTILES, B], f16, tag="xal")

            # persistent small constants, loaded early
            b1t = stage.tile([128, NCHUNK], f32, tag="b1t")
            nc.sync.dma_start(b1t[:], b1c_d[:])
            ident = stage.tile([BLOC, BLOC], f32, tag="ident")
            make_identity(nc, ident[:])

            curb = stage.tile([128, NCHUNK * BLOC], f32, tag="curb")

            # hidden quarters (2 chunks each): shared x stationaries across
            # the quarter's 2 chunks keep LDWEIGHTS (2 per 6 matmuls) hidden,
            # while each quarter's ReduceScatter overlaps the next quarter's
            # matmuls.
            rs_qs = []
            for qd in range(4):
                wh = wpool.tile([128, KTILES, 256], f16, tag="wh")
                wl = wpool.tile([128, KTILES, 256], f16, tag="wl")
                if qd == 0:
                    # stage a minimal first x piece + first weight chunk so
                    # the PE can start as early as possible.
                    nc.sync.dma_start(xall_h[:, 0:4, :], xth_d[:, 0:4, :])
                    nc.sync.dma_start(xall_l[:, 0:4, :], xtl_d[:, 0:4, :])
                    nc.sync.dma_start(wh[:, 0:4, :], w1h_d[qd][:, 0:4, :])
                    nc.sync.dma_start(wl[:, 0:4, :], w1l_d[qd][:, 0:4, :])
                    nc.sync.dma_start(xall_h[:, 4:12, :], xth_d[:, 4:12, :])
                    nc.sync.dma_start(xall_l[:, 4:12, :], xtl_d[:, 4:12, :])
                    nc.sync.dma_start(wh[:, 4:12, :], w1h_d[qd][:, 4:12, :])
                    nc.sync.dma_start(wl[:, 4:12, :], w1l_d[qd][:, 4:12, :])
                    for q in range(3):
                        ktq = slice(12 + q * 8, min(12 + (q + 1) * 8, 32))
                        nc.sync.dma_start(xall_h[:, ktq, :], xth_d[:, ktq, :])
                        nc.sync.dma_start(xall_l[:, ktq, :], xtl_d[:, ktq, :])
                        nc.sync.dma_start(wh[:, ktq, :], w1h_d[qd][:, ktq, :])
                        nc.sync.dma_start(wl[:, ktq, :], w1l_d[qd][:, ktq, :])
                else:
                    # chunked weight DMAs: matmuls for kt-range r start as
                    # soon as chunk r lands, and the WAR release on the
                    # rotated buffer is finer-grained.
                    for r in range(4):
                        ktr = slice(r * 8, (r + 1) * 8)
                        nc.sync.dma_start(wh[:, ktr, :], w1h_d[qd][:, ktr, :])
                        nc.sync.dma_start(wl[:, ktr, :], w1l_d[qd][:, ktr, :])

                # both mb accumulators packed into one PSUM bank-tile
                psq = psA.tile([128, 2, 256], f32, tag="psq")
                ps = [psq[:, mb, :] for mb in range(2)]
                for kt in range(KTILES):
                    for mb in range(2):
                        xh_s = xall_h[:, kt, mb * 128:(mb + 1) * 128]
                        xl_s = xall_l[:, kt, mb * 128:(mb + 1) * 128]
                        # start=True clears has_written for the WHOLE bank, so
                        # only the very first matmul into the packed bank-tile
                        # may carry it; the other region's first write lands on
                        # cleared bits and overwrites per the per-element rule.
                        nc.tensor.matmul(ps[mb], xh_s, wl[:, kt, :],
                                         start=(kt == 0 and mb == 0),
                                         stop=False)
                        nc.tensor.matmul(ps[mb], xh_s, wh[:, kt, :],
                                         start=False, stop=False)
                        nc.tensor.matmul(ps[mb], xl_s, wh[:, kt, :],
                                         start=False, stop=(kt == KTILES - 1))

                # drain (scale back) + store partial to DRAM, batch-major
                partial = dram.tile([B, 256], f32, tag=f"pa{qd}", name=f"partial{qd}")
                for mb in range(2):
                    csb = cspool.tile([128, 256], f32, tag=f"cs{mb}",
                                      name=f"cs{mb}_{qd}")
                    nc.scalar.activation(csb[:], ps[mb],
                                         mybir.ActivationFunctionType.Copy,
                                         scale=1.0 / W1SCALE)
                    # scalar queue: keeps the sync queue a pure input-prefetch
                    # stream (in-order queues head-of-line block otherwise)
                    nc.scalar.dma_start(partial[mb * 128:(mb + 1) * 128, :], csb[:])

                rs_q = dram.tile([BLOC, 256], f32, tag=f"rs{qd}", name=f"rs{qd}")
                if sim:
                    nc.sync.dma_start(rs_q[:], partial[0:BLOC, :])
                else:
                    nc.gpsimd.collective_compute(
                        "ReduceScatter",
                        mybir.AluOpType.add,
                        replica_groups=[list(range(N_CORES))],
                        ins=[partial.opt()],
                        outs=[rs_q.opt()],
                    )
                rs_qs.append(rs_q)

            # RS-dependent work LAST, so nothing that waits on a collective
            # sits ahead of later quarters' matmuls in any in-order queue.
            # rsb loads ride the scalar queue after all partial stores;
            # b1 folds in via a DVE tensor_scalar with a [P,1] scalar AP.
            for qd in range(4):
                rsb = rsbpool.tile([BLOC, 256], f32, tag="rsb",
                                   name=f"rsb{qd}")
                nc.scalar.dma_start(rsb[:], rs_qs[qd][:])
                for cl in range(2):
                    c = 2 * qd + cl
                    pt = psT.tile([128, BLOC], f32, tag="pst",
                                  name=f"pst{qd}{cl}")
                    nc.tensor.matmul(pt[:], rsb[:, cl * 128:(cl + 1) * 128],
                                     ident[:], start=True, stop=True,
                                     is_transpose=True)
                    nc.vector.tensor_scalar(
                        curb[:, c * BLOC:(c + 1) * BLOC], pt[:],
                        b1t[:, c:c + 1], None, mybir.AluOpType.add)

            if dbg:
                nc.sync.dma_start(curdbg_d[:], curb[:])

            # ---------------- scan constants
            w2hi_t = stage.tile([128, NCHUNK, NO], bf16, tag="w2hi")
            nc.sync.dma_start(w2hi_t[:], w2hi_d[:])
            w2lo_t = stage.tile([128, NCHUNK, NO], bf16, tag="w2lo")
            nc.sync.dma_start(w2lo_t[:], w2lo_d[:])
            b2c_t = stage.tile([NO, 1], f32, tag="b2c")
            nc.sync.dma_start(b2c_t[:], b2c_d[:])
            biasm1 = stage.tile([128, 1], f32, tag="bm1")
            nc.vector.memset(biasm1[:], -1.0)
            zeros_t = stage.tile([128, NCHUNK * BLOC], f32, tag="zeros")
            nc.vector.memset(zeros_t[:], 0.0)
            z5 = stage.tile([NO, BLOC], f32, tag="z5")
            nc.vector.memset(z5[:], 0.0)
            mem2rec = stage.tile([NO, T, BLOC], f32, tag="m2r")
            c2tiles = [None] * NGROUP  # rotating per-group cur2 tiles

            def d_step(dt):
                """Layer-2 LIF step dt (0-based) on DVE, [5, 32] layout.

                b2eff is pre-folded into c2 by the drain's ACT bias, so the
                plain 1-scalar LIF op suffices (the [P,1] C1 variant costs
                ~+60ns/op in AP latch setup).
                """
                in0 = z5[:] if dt == 0 else mem2rec[:, dt - 1, :]
                nc.vector._custom_dve(
                    LIF,
                    out=mem2rec[:, dt, :],
                    in0=in0,
                    in1=c2tiles[dt // G][:, dt % G, :],
                    s0=BETA,
                )

            def out_block(t0, t1):
                # DVE, not GPSIMD: a [5, N] gpsimd tensor_scalar measures
                # ~13.7us and stalls concurrent DVE work via the shared SBUF
                # port; the DVE op is ~0.7us.
                sblk = spool.tile([NO, (OBLK + DLAG) * G, BLOC], f32, tag="sblk")
                nc.vector.tensor_scalar(
                    sblk[:, 0:t1 - t0, :], mem2rec[:, t0:t1, :], 1.0, None,
                    mybir.AluOpType.is_gt)
                nc.sync.dma_start(mem2_d[:, t0:t1, :], mem2rec[:, t0:t1, :])
                nc.sync.dma_start(spk2_d[:, t0:t1, :], sblk[:, 0:t1 - t0, :])

            # ---------------- phase B/C/D: fused scan
            # The first SPLIT_G groups run the layer-1 LIF column-split:
            # chunks 0-5 (quarters 1-3 of curb, ready first) scan ahead on
            # DVE while quarter 4's ReduceScatter is still in flight; chunks
            # 6-7 catch up afterwards. Group-end work (PE contraction, drain,
            # layer-2) for those groups is deferred until both halves exist.
            SPLIT_G = 12
            ASZ = 6 * BLOC   # chunks 0-5
            BSZ = 2 * BLOC   # chunks 6-7

            def group_tail(gi, gt):
                """PE contraction + cur2 drain for group gi."""
                pc = psC.tile([NO, G, BLOC], f32, tag="pc", name=f"pc{gi}")
                for c2 in range(NCHUNK):
                    rhs = gt[:, :, c2, :]
                    nc.tensor.matmul(pc[:], w2hi_t[:, c2, :], rhs,
                                     start=(c2 == 0), stop=False)
                    nc.tensor.matmul(pc[:], w2lo_t[:, c2, :], rhs,
                                     start=False, stop=(c2 == NCHUNK - 1))
                # drain, folding b2eff in via the per-partition ACT bias
                c2g = cpool.tile([NO, G, BLOC], f32, tag="c2g", name=f"c2g{gi}")
                c2tiles[gi] = c2g
                nc.scalar.activation(
                    c2g[:], pc[:], mybir.ActivationFunctionType.Identity,
                    bias=b2c_t[:], scale=1.0)
                if dbg:
                    nc.sync.dma_start(c2dbg_d[:, gi * G:(gi + 1) * G, :],
                                      c2g[:])

            # phase 1: A-columns scan-ahead for the first SPLIT_G groups
            memA = zeros_t[:, 0:ASZ]
            gts = [None] * SPLIT_G
            mgAs = [None] * SPLIT_G
            for t in range(1, SPLIT_G * G + 1):
                gi, sl = (t - 1) // G, (t - 1) % G
                if sl == 0:
                    mgAs[gi] = mpoolA.tile([128, G, ASZ], f32, tag="mgA",
                                           name=f"mgA{gi}")
                out_v = mgAs[gi][:, sl, :]
                nc.vector._custom_dve(LIF, out=out_v, in0=memA,
                                      in1=curb[:, 0:ASZ], s0=BETA)
                memA = out_v
                if sl == G - 1:
                    gts[gi] = gpool.tile([128, G, NCHUNK, BLOC], bf16,
                                         tag="gt", name=f"gt{gi}")
                    nc.scalar.activation(
                        gts[gi][:, :, 0:6, :],
                        mgAs[gi][:].rearrange("p g (c b) -> p g c b", b=BLOC),
                        mybir.ActivationFunctionType.Sign,
                        bias=biasm1[:], scale=1.0,
                    )

            # phase 2 prologue: B-columns catch up; full group tails follow
            memB = zeros_t[:, ASZ:]
            mgB = None
            for t in range(1, SPLIT_G * G + 1):
                gi, sl = (t - 1) // G, (t - 1) % G
                if sl == 0:
                    mgB = mpoolB.tile([128, G, BSZ], f32, tag="mgB")
                out_v = mgB[:, sl, :]
                nc.vector._custom_dve(LIF, out=out_v, in0=memB,
                                      in1=curb[:, ASZ:], s0=BETA)
                memB = out_v
                if sl == G - 1:
                    nc.scalar.activation(
                        gts[gi][:, :, 6:8, :],
                        mgB[:].rearrange("p g (c b) -> p g c b", b=BLOC),
                        mybir.ActivationFunctionType.Sign,
                        bias=biasm1[:], scale=1.0,
                    )
                    group_tail(gi, gts[gi])
                    gts[gi] = None

            # merge the split state into a unified tile for step SPLIT_G*G
            mg = mpool.tile([128, G, NCHUNK * BLOC], f32, tag="mg")
            nc.vector.tensor_copy(mg[:, G - 1, 0:ASZ],
                                  mgAs[SPLIT_G - 1][:, G - 1, :])
            nc.vector.tensor_copy(mg[:, G - 1, ASZ:], memB)
            mem_prev = mg[:, G - 1, :]

            out_done = 0
            d_done = 0
            for t in range(SPLIT_G * G + 1, T + 1):
                gi, sl = (t - 1) // G, (t - 1) % G
                if sl == 0:
                    mg = mpool.tile([128, G, NCHUNK * BLOC], f32, tag="mg")
                out_v = mg[:, sl, :]
                nc.vector._custom_dve(LIF, out=out_v, in0=mem_prev,
                                      in1=curb[:], s0=BETA)
                mem_prev = out_v
                if sl == G - 1:
                    # batched sign for the whole group -> g in {-1,+1} bf16
                    gt = gpool.tile([128, G, NCHUNK, BLOC], bf16, tag="gt",
                                    name=f"gt{gi}")
                    nc.scalar.activation(
                        gt[:].rearrange("p g c b -> p (g c b)"),
                        mg[:].rearrange("p g f -> p (g f)"),
                        mybir.ActivationFunctionType.Sign,
                        bias=biasm1[:], scale=1.0,
                    )
                    group_tail(gi, gt)
                    # layer-2 LIF, lagged DLAG groups (flushes the deferred
                    # split-phase groups on the first unified group)
                    if gi >= DLAG:
                        for dg in range(d_done, gi - DLAG + 1):
                            for dt in range(dg * G, (dg + 1) * G):
                                d_step(dt)
                        d_done = gi - DLAG + 1
                    # spk2 + output DMA in OBLK-group blocks, cursor-based
                    while d_done * G - out_done >= OBLK * G:
                        out_block(out_done, out_done + OBLK * G)
                        out_done += OBLK * G
                    if gi == NGROUP - 1 and d_done * G > out_done:
                        # flush everything already d-stepped
                        out_block(out_done, d_done * G)
                        out_done = d_done * G

            # ---------------- tail
            for dt in range((NGROUP - DLAG) * G, T):
                d_step(dt)
            out_block(out_done, T)

    nc.compile()
    _PROGRAMS[key] = (nc, LIF, LIF2)
    return _PROGRAMS[key]


# ---------------------------------------------------------------- host prep
def _prep_inputs(x, W1, b1, W2, b2):
    f32 = np.float32
    f16 = np.float16
    x_flat = np.ascontiguousarray(x.reshape(B, -1).astype(f32, copy=False))
    xT = np.zeros((KPAD, B), f32)
    xT[:NIN] = x_flat.T
    xTh = xT.astype(f16)
    xTl = (xT - xTh.astype(f32)).astype(f16)
    w1T = np.zeros((KPAD, HPAD), f32)
    w1T[:NIN, :NH] = W1.astype(f32, copy=False).T * W1SCALE
    w1Th = w1T.astype(f16)
    w1Tl = (w1T - w1Th.astype(f32)).astype(f16)
    b1p = np.full(HPAD, -10.0, f32)
    b1p[:NH] = b1
    b1c = np.ascontiguousarray(b1p.reshape(NCHUNK, 128).T)          # [128, 8]
    W2e = np.zeros((HPAD, NO), f32)
    W2e[:NH] = 0.5 * W2.astype(f32, copy=False).T
    w2stack = np.ascontiguousarray(W2e.reshape(NCHUNK, 128, NO).transpose(1, 0, 2))
    w2hi = w2stack.astype(ml_dtypes.bfloat16)
    w2lo = (w2stack - w2hi.astype(f32)).astype(ml_dtypes.bfloat16)
    b2eff = (b2.astype(f32) + 0.5 * W2.astype(f32).sum(axis=1)).reshape(NO, 1)
    b2eff = np.ascontiguousarray(b2eff.astype(f32))

    in_maps = []
    for cidx in range(N_CORES):
        ksl = slice(cidx * KC, (cidx + 1) * KC)
        xh = xTh[ksl].reshape(KTILES, 128, B).transpose(1, 0, 2)
        xl = xTl[ksl].reshape(KTILES, 128, B).transpose(1, 0, 2)
        # [KC, HPAD] -> [KTILES, 128, 4, 256] -> [4, 128, KTILES, 256]
        wh = w1Th[ksl].reshape(KTILES, 128, 4, 256).transpose(2, 1, 0, 3)
        wl = w1Tl[ksl].reshape(KTILES, 128, 4, 256).transpose(2, 1, 0, 3)
        in_maps.append({
            "xth": np.ascontiguousarray(xh),
            "xtl": np.ascontiguousarray(xl),
            "w1h": np.ascontiguousarray(wh),
            "w1l": np.ascontiguousarray(wl),
            "b1c": b1c,
            "w2hi": w2hi,
            "w2lo": w2lo,
            "b2c": b2eff,
        })
    return in_maps


def _gather(results):
    spk_parts, mem_parts = [], []
    for r in results:
        mem_parts.append(r["mem2rec"].transpose(1, 2, 0))  # [T, BLOC, NO]
        spk_parts.append(r["spk2rec"].transpose(1, 2, 0))
    mem2 = np.concatenate(mem_parts, axis=1).astype(np.float32)  # [200, 256, 5]
    spk2 = np.concatenate(spk_parts, axis=1).astype(np.float32)
    return spk2, mem2


def run_raw(inputs, dbg=False, **kwargs):
    """Build+run; returns BassKernelResults (for profiling from test.py)."""
    from concourse.bass_utils import run_bass_kernel_spmd

    nc, _, _ = _build_program(dbg=dbg)
    in_maps = _prep_inputs(**inputs)
    return run_bass_kernel_spmd(nc, in_maps, core_ids=list(range(N_CORES)), **kwargs)


def kernel(x, W1, b1, W2, b2):
    res = run_raw(dict(x=x, W1=W1, b1=b1, W2=W2, b2=b2))
    return _gather(res.results)


if __name__ == "__main__":
    rng = np.random.default_rng(0)
    ins = {
        "x": rng.standard_normal((B, 2, 80, 200)).astype(np.float32),
        "W1": rng.uniform(-1, 1, (NH, NIN)).astype(np.float32) / np.sqrt(NIN),
        "b1": rng.uniform(-1, 1, NH).astype(np.float32) / np.sqrt(NIN),
        "W2": rng.uniform(-1, 1, (NO, NH)).astype(np.float32) / np.sqrt(NH),
        "b2": rng.uniform(-1, 1, NO).astype(np.float32) / np.sqrt(NH),
    }
    spk2, mem2 = kernel(**ins)
    print("shapes:", spk2.shape, mem2.shape, spk2.dtype, mem2.dtype)
    print("spk2 mean:", spk2.mean(), "mem2 std:", mem2.std())
